# revision 27
# baseline (speedup 1.0000x reference)
"""DropConnect forward kernel for Trainium2 (8 NeuronCores, Bass/Tile).

y[n,o] = (sum_k x[n,k] * weight[k,o] * w_mask[n,k,o] + bias[o]*b_mask[n,o]) * 2

Data-parallel over the batch N=256 -> 32 samples per core. The dominant
cost is streaming the 0/1 w_mask (4 MiB/sample fp32) and elementwise
multiplying it into the weights. Measured on HW, GPSIMD tensor ops
degrade concurrent DVE ops ~4x (SBUF arbitration), so the DVE is the
ONLY multiply engine; the other engines reduce DMA bytes and drain:

  route 'b' (bf16 slab, DVE mul)            : DVE ~4.4us, DMA 2 MiB
  route 'c' (fp8 slab, ACT cvt -> DVE mul)  : ACT ~7.3us, DVE ~4.4us, 1 MiB
  route 'e' (u16 bitpack, DVE and+mul)      : DVE ~8.8us, DMA 0.125 MiB

All three per-engine totals and the DMA aggregate land at ~150us.

Unified layout for every route: k = 8p + j (p = SBUF partition), o =
half*512 + w, slab free index f = b*512 + w with b = half*8 + j. The
bf16 weight tile wpt[p, f] = 2*W[8p+(b&7), (b>>3)*512+w] serves all
routes and the PE schedule is identical: per sample 16 matmuls of
[128,512] (moving = product chunk, stationary = x column), PSUM-
accumulated over j per half (tile_position column groups 0/32).
Elementwise work is done in 2 chunks of [128,4096] (= one half each) to
amortize instruction overheads. Route 'e' extracts mask bits with a
tensor_tensor bitwise_and (stride-0 repeat AP on the packed words, all
operands 2-byte so the DVE 2x mode still applies); the 2^b factor it
leaves on the products is cancelled by a per-b pre-scaled x stationary
(exact powers of two).

Samples are PSUM-batched in groups of 4 (one [64,2048] PSUM tile = 4
fp32 banks, slot i = n%4); each group is drained by two batched ACT
copies ([1,2048]) and scattered into the output tile by two small DMAs.
DMA rings: 'b' slabs on SP, 'c'/'e' slabs on the Pool ring (the Pool
sequencer issues DGE cheaply and runs no tensor ops), constants/drain
scatters/output on the ACT ring. The bias epilogue runs on GPSIMD at
the very end where it cannot interfere with the DVE.
"""

import sys

for _p in ("/opt/trn_rl_repo",):
    if _p not in sys.path:
        sys.path.insert(0, _p)

import numpy as np

import concourse.bass as bass
from concourse.bass import AP
import concourse.tile as tile
from concourse import bacc, mybir
from concourse.alu_op_type import AluOpType
from concourse.bass_utils import run_bass_kernel_spmd

N_CORES = 8
NS = 32            # samples per core
D = 1024           # in_dim == out_dim
P = 128            # SBUF partitions
J = D // P         # 8 k-subtiles interleaved per partition row
F = J * D          # 8192 free elements per full slab
NH = 512           # PSUM half width (one fp32 bank)
NCHUNK = 4         # product chunks per sample
CW = F // NCHUNK   # 2048 elements per chunk (4 b-blocks)
NWORD = 512        # u16 words per partition in a bitpacked slab

FP32 = mybir.dt.float32
BF16 = mybir.dt.bfloat16
FP8 = mybir.dt.float8e4
U8 = mybir.dt.uint8
U16 = mybir.dt.uint16

# Route assignment for the 32 samples of every core: 'b' first so the
# DVE starts as soon as the first sync-ring slab lands, then strict b/c
# alternation; the DVE-heavy 'e' samples sit late but off the last
# group. Counts: b=15, c=15, e=2.
ROUTES32 = list(
    "b b c b c "
    "b c b c b "
    "c b c b c "
    "b c b c b "
    "c e b c c "
    "c c e c c "
    "b b".split()
)
assert len(ROUTES32) == 32

# test.py pokes this to get a traced run; the grading path never touches it.
TRACE = {"trace": False, "last_result": None, "trace_kwargs": {}}


def _build_nc(ns: int = NS, routes=None):
    if routes is None:
        routes = ROUTES32 if ns == NS else (["b", "c", "e", "c"] * ns)[:ns]
    assert len(routes) == ns and ns % 4 == 0
    nb = routes.count("b")
    nc8 = routes.count("c")
    ne = routes.count("e")
    assert nb + nc8 + ne == ns

    nc = bacc.Bacc("TRN2", target_bir_lowering=False, debug=False)

    wmb = nc.declare_dram_parameter("wmb", [max(nb, 1), P, F], BF16, isOutput=False)
    wmc = nc.declare_dram_parameter("wmc", [max(nc8, 1), P, F], FP8, isOutput=False)
    wme = nc.declare_dram_parameter("wme", [max(ne, 1), P, NWORD], U16, isOutput=False)
    wp = nc.declare_dram_parameter("wp", [P, F], BF16, isOutput=False)
    pt = nc.declare_dram_parameter("pt", [P, F], U16, isOutput=False)
    xt = nc.declare_dram_parameter("xt", [P, J * ns], BF16, isOutput=False)
    xp = nc.declare_dram_parameter("xp", [P, 2 * J * ns], BF16, isOutput=False)
    bm = nc.declare_dram_parameter("bm", [ns, D], FP32, isOutput=False)
    b2 = nc.declare_dram_parameter("b2", [ns, D], FP32, isOutput=False)
    y = nc.declare_dram_parameter("y", [ns, D], FP32, isOutput=True)

    with tile.TileContext(nc) as tc:
        with (
            tc.tile_pool(name="const", bufs=1) as cpool,
            tc.tile_pool(name="bslab", bufs=3) as bpool,
            tc.tile_pool(name="cslab", bufs=3) as cspool,

            tc.tile_pool(name="conv", bufs=6) as cvpool,
            tc.tile_pool(name="andp", bufs=2) as anpool,
            tc.tile_pool(name="prod", bufs=8) as prpool,
            tc.tile_pool(name="stage", bufs=2) as stpool,
            tc.tile_pool(name="psum", bufs=2, space=bass.MemorySpace.PSUM) as ppool,
        ):
            # Constants: wp0 + x stationaries on the ACT ring up front; the
            # remaining weight chunks are spread across the sync and Pool
            # rings right behind the first slab of each (the scalar ring
            # alone delivered them too late and starved the DVE during the
            # pipeline fill). Bit patterns go last (first 'e' sample is
            # late).
            wpt = []
            patt = []
            for q in range(NCHUNK):
                wq = cpool.tile([P, CW], BF16, tag=f"wp{q}")
                wpt.append(wq)
                pq = cpool.tile([P, CW], U16, tag=f"pt{q}")
                patt.append(pq)
            nc.scalar.dma_start(out=wpt[0][:], in_=wp[:, 0:CW])
            nc.sync.dma_start(out=wpt[1][:], in_=wp[:, CW : 2 * CW])
            nc.gpsimd.dma_start(out=wpt[2][:], in_=wp[:, 2 * CW : 3 * CW])
            nc.gpsimd.dma_start(out=wpt[3][:], in_=wp[:, 3 * CW : 4 * CW])
            xtt = cpool.tile([P, J * ns], BF16, tag="xt")
            nc.scalar.dma_start(out=xtt[:], in_=xt[:])
            xpt = cpool.tile([P, 2 * J * ns], BF16, tag="xp")
            nc.scalar.dma_start(out=xpt[:], in_=xp[:])
            # Bit patterns + the tiny bitpacked slabs land up front: the
            # scheduler hoists the 'e' ANDs into DVE gaps, which only works
            # if their inputs are already resident.
            for q in range(NCHUNK):
                nc.scalar.dma_start(out=patt[q][:], in_=pt[:, q * CW : (q + 1) * CW])
            eslabs = []
            for s in range(ne):
                es = cpool.tile([P, NWORD], U16, tag=f"eslab{s}")
                nc.scalar.dma_start(out=es[:], in_=wme[s, :, :])
                eslabs.append(es)
            bmt = cpool.tile([ns, D], FP32, tag="bm")
            nc.scalar.dma_start(out=bmt[:], in_=bm[:])
            b2t = cpool.tile([ns, D], FP32, tag="b2")
            nc.scalar.dma_start(out=b2t[:], in_=b2[:])
            yt = cpool.tile([ns, D], FP32, tag="y")
            bbt = cpool.tile([ns, D], FP32, tag="bb")
            # bb[n,o] = 2*bias[o]*b_mask[n,o] on GPSIMD early, during the
            # pipeline fill where it cannot slow the DVE down.
            nc.gpsimd.tensor_mul(bbt[:], bmt[:], b2t[:])

            ib = ic = ie = 0
            for g in range(ns // 4):
                ps = ppool.tile([64, 4 * NH], FP32, tag="ps")
                for i in range(4):
                    n = 4 * g + i
                    r = routes[n]
                    if r == "b":
                        slab = bpool.tile([P, F], BF16, tag="bslab")
                        nc.sync.dma_start(out=slab[:], in_=wmb[ib, :, :])
                        ib += 1
                    elif r == "c":
                        slab = cspool.tile([P, F], FP8, tag="cslab")
                        nc.gpsimd.dma_start(out=slab[:], in_=wmc[ic, :, :])
                        ic += 1
                    else:
                        slab = eslabs[ie]
                        ie += 1

                    for q in range(NCHUNK):
                        off = q * CW
                        pr = prpool.tile([P, CW], BF16, tag="prod")
                        if r == "b":
                            nc.vector.tensor_mul(
                                pr[:], slab[:, off : off + CW], wpt[q][:]
                            )
                        elif r == "c":
                            cv = cvpool.tile([P, CW], BF16, tag="conv")
                            nc.scalar.copy(cv[:], slab[:, off : off + CW])
                            nc.vector.tensor_mul(pr[:], cv[:], wpt[q][:])
                        else:
                            an = anpool.tile([P, CW], U16, tag="andp")
                            src = slab[:, :]
                            rep = AP(
                                src.tensor,
                                src.offset,
                                [list(src.ap[0]), [0, CW // NWORD], [1, NWORD]],
                            )
                            nc.vector.tensor_tensor(
                                out=an[:], in0=rep, in1=patt[q][:],
                                op=AluOpType.bitwise_and,
                            )
                            nc.vector.tensor_mul(pr[:], an[:], wpt[q][:])

                        for bb in range(CW // NH):
                            b = q * (CW // NH) + bb
                            half = b >> 3
                            j = b & 7
                            if r == "e":
                                lhsT = xpt[:, b * ns + n : b * ns + n + 1]
                            else:
                                lhsT = xtt[:, j * ns + n : j * ns + n + 1]
                            nc.tensor.matmul(
                                ps[32 * half : 32 * half + 1, NH * i : NH * (i + 1)],
                                lhsT,
                                pr[:, NH * bb : NH * (bb + 1)],
                                start=(j == 0),
                                stop=(j == 7),
                                tile_position=(0, 32 * half),
                            )

                # Batched drain of the 4-sample group: one ACT copy per
                # half, then a small DMA scatters 4 rows into yt.
                for half in range(2):
                    st = stpool.tile([1, 4 * NH], FP32, tag="stage")
                    nc.scalar.copy(st[0:1, :], ps[32 * half : 32 * half + 1, :])
                    yd = yt[:, :]
                    dst = AP(
                        yd.tensor,
                        yd.offset + 4 * g * int(yd.ap[0][0]) + half * NH,
                        [[int(yd.ap[0][0]), 4], [1, NH]],
                    )
                    nc.scalar.dma_start(out=dst, in_=st[0:1, :], single_packet=True)

            # Final bias add on the DVE, which is idle by the time the last
            # scatter lands (bbt was computed during the pipeline fill).
            nc.vector.tensor_add(yt[:], yt[:], bbt[:])
            nc.scalar.dma_start(out=y[:], in_=yt[:])

    nc.compile()
    return nc


def _host_prep(x, weight, bias, w_mask, b_mask, ns=NS, routes=None, n_cores=N_CORES):
    """Shard + lay out inputs. Layout/dtype-encoding only (plus exact *2)."""
    if routes is None:
        routes = ROUTES32 if ns == NS else (["b", "c", "e", "c"] * ns)[:ns]
    x = np.ascontiguousarray(x, dtype=np.float32)
    weight = np.ascontiguousarray(weight, dtype=np.float32)
    bias = np.ascontiguousarray(bias, dtype=np.float32)
    b_mask = np.ascontiguousarray(b_mask, dtype=np.float32)

    import ml_dtypes

    # Unified slab order: f = b*512 + w, b = half*8 + j, k = 8p+j,
    # o = half*512 + w.
    def to_slab_order(m):  # m: [..., 1024(k), 1024(o)] -> [..., 128, 8192]
        lead = m.shape[:-2]
        t = m.reshape(*lead, P, J, 2, NH)          # [..., p, j, half, w]
        t = np.moveaxis(t, -2, -3)                 # [..., p, half, j, w]
        return np.ascontiguousarray(t.reshape(*lead, P, F))

    wpv = to_slab_order(2.0 * weight).astype(ml_dtypes.bfloat16)
    ptv = np.broadcast_to(
        (1 << (np.arange(F, dtype=np.uint32) // NH)).astype(np.uint16)[None, :],
        (P, F),
    ).copy()
    b2 = np.tile((2.0 * bias)[None, :], (ns, 1)).astype(np.float32)

    xb = x.astype(ml_dtypes.bfloat16).astype(np.float32)
    # 2^-b scale for the bitpacked route (exact in bf16).
    scl = np.float32(2.0) ** (-np.arange(2 * J, dtype=np.float32))

    in_maps = []
    for c in range(n_cores):
        sl = slice(c * ns, (c + 1) * ns)
        mro = to_slab_order(w_mask[sl])            # [ns, 128, 8192] f32 0/1
        xc = xb[sl]                                # [ns, 1024] f32 (bf16 vals)

        bl, cl, el = [], [], []
        for n in range(ns):
            r = routes[n]
            if r == "b":
                bl.append(mro[n].astype(ml_dtypes.bfloat16))
            elif r == "c":
                cl.append(mro[n].astype(ml_dtypes.float8_e4m3))
            else:
                bits = mro[n].reshape(P, 2 * J, NWORD).astype(np.uint32)
                words = np.zeros((P, NWORD), np.uint32)
                for b in range(2 * J):
                    words |= bits[:, b, :] << b
                el.append(words.astype(np.uint16))

        def stack(lst, shape, dt):
            if not lst:
                return np.zeros((1,) + shape, dt)
            return np.ascontiguousarray(np.stack(lst))

        # x stationaries: xt[p, j*ns+n] = x[n, 8p+j]; xp adds the 2^-b scale
        # with j = b&7.
        xtr = xc.T.reshape(P, J, ns)               # [p, j, n]
        xtv = np.ascontiguousarray(
            xtr.reshape(P, J * ns)
        ).astype(ml_dtypes.bfloat16)
        xpv = np.ascontiguousarray(
            (np.concatenate([xtr, xtr], axis=1)    # [p, 16(b), n]
             * scl[None, :, None]).reshape(P, 2 * J * ns)
        ).astype(ml_dtypes.bfloat16)

        in_maps.append(
            {
                "wmb": stack(bl, (P, F), ml_dtypes.bfloat16),
                "wmc": stack(cl, (P, F), ml_dtypes.float8_e4m3),
                "wme": stack(el, (P, NWORD), np.uint16),
                "wp": wpv,
                "pt": ptv,
                "xt": xtv,
                "xp": xpv,
                "bm": np.ascontiguousarray(b_mask[sl]),
                "b2": b2,
            }
        )
    return in_maps


def kernel(x, weight, bias, w_mask, b_mask):
    # accept jax or numpy arrays
    x, weight, bias, w_mask, b_mask = (
        np.asarray(a) for a in (x, weight, bias, w_mask, b_mask)
    )
    in_maps = _host_prep(x, weight, bias, w_mask, b_mask)
    nc = _build_nc()
    res = run_bass_kernel_spmd(
        nc,
        in_maps,
        core_ids=list(range(N_CORES)),
        trace=TRACE["trace"],
        **TRACE["trace_kwargs"],
    )
    TRACE["last_result"] = res
    out = np.concatenate([res.results[c]["y"] for c in range(N_CORES)], axis=0)
    return out.astype(np.float32, copy=False)
